# revision 1
# baseline (speedup 1.0000x reference)
"""Trainium2 Bass kernel for nn_EquivariantUpdateLayer (GNN message passing), v2.

Edge-parallel across 8 NeuronCores with FIXED destination windows:
core c owns nodes [c*6272, (c+1)*6272); each window = 112 consecutive nodes
(56 windows/core). Per-window edge slots are split into a lo-section
(ej < 32768) and hi-section, each padded to a global per-input capacity
(CAP_LO/CAP_HI, multiples of 128) so the whole program is input-shape-fixed.

Device pipeline (H-layout [hidden, edges]):
- pretransform: B = h @ W1b (full graph) and A = h_own @ W1a (own rows) via
  dma_start_transpose loads + matmuls, written to DRAM scratch.
- per window: dma_gather(transpose=True) fetches B rows for the window's
  edges -> BgT [H, CAP]; z1 = A_ext^T @ onehotT (host-built one-hot with a
  d2 row, applies A[ei] + d2*w1c) + I^T @ BgT (PSUM-accumulated); silu+b1.
- LN via selector-matmul stats into one [10, 512] PSUM pane per window
  (rows 0..NTW-1 = sum(s1), NTW..2NTW-1 = sum(s1^2)), row-math on DVE,
  rstd broadcast via a DRAM roundtrip read with a partition-replicating AP.
- z2 = W2g^T (s1*rstd) + negu^T (mu*rstd) (gamma/beta folded into W2g/b2p),
  silu; z3 = W3^T s2, silu; per-chunk W4 matmuls -> scale; vec = dx*scale;
  per-chunk one-hot pane matmuls aggregate into [112, 3]; + x; affine out.
Host only sorts/pads/builds index+one-hot streams and reassembles windows.
"""
import hashlib
import numpy as np
import ml_dtypes

bf16 = ml_dtypes.bfloat16
f32 = np.float32

NCORES = 8
N = 50000
E_TOT = 800000
H = 128
W = 112            # nodes per window
NWIN = 56          # windows per core
OWN = W * NWIN     # 6272 nodes per core (128-aligned)
NP_FULL = 50176    # padded global rows (392 * 128)
LO = 32768         # B-table split row
EPS = 1e-5
CHUNK = 128
TPT = 4            # chunks per tile
TRACE = False
TRACE_KW = {}
DBG = False
DBGW = 3


# ---------------------------------------------------------------- host prep --

def _prepare(h, x, e, dx, d2):
    ei = np.asarray(e[0], np.int64)
    ej = np.asarray(e[1], np.int64)
    dx = np.asarray(dx, f32)
    d2 = np.asarray(d2, f32).reshape(-1)

    core = ei // OWN
    win = (ei % OWN) // W
    rel = ei % W
    is_hi = ej >= LO

    # per (core, window) lo/hi counts -> global capacities
    gw = core * NWIN + win
    n_lo = np.bincount(gw[~is_hi], minlength=NCORES * NWIN)
    n_hi = np.bincount(gw[is_hi], minlength=NCORES * NWIN)
    cap_lo = int(-(-n_lo.max() // CHUNK) * CHUNK)
    cap_hi = int(-(-n_hi.max() // CHUNK) * CHUNK)
    cap = cap_lo + cap_hi
    nch = cap // CHUNK
    nchl = cap_lo // CHUNK

    # slot assignment: order edges by (core, window, is_hi), then place
    order = np.lexsort((is_hi, win, core))
    ei_s, ej_s, rel_s = ei[order], ej[order], rel[order]
    core_s, win_s, hi_s = core[order], win[order], is_hi[order]
    dx_s, d2_s = dx[order], d2[order]
    # rank within each (core, window, section)
    key = (core_s * NWIN + win_s) * 2 + hi_s
    starts = np.r_[0, 1 + np.flatnonzero(np.diff(key))]
    counts = np.diff(np.r_[starts, len(key)])
    rank = np.arange(len(key)) - np.repeat(starts, counts)
    slot = win_s * cap + np.where(hi_s, cap_lo, 0) + rank

    data = []
    for c in range(NCORES):
        m = core_s == c
        sl = slot[m]
        nslots = NWIN * cap
        # gather idx streams (pad -> row 0)
        idx_lo = np.zeros(NWIN * cap_lo, np.int64)
        idx_hi = np.zeros(NWIN * cap_hi, np.int64)
        wn, off = np.divmod(sl, cap)
        in_hi = off >= cap_lo
        idx_lo[(wn[~in_hi] * cap_lo + off[~in_hi])] = ej_s[m][~in_hi]
        idx_hi[(wn[in_hi] * cap_hi + (off[in_hi] - cap_lo))] = \
            ej_s[m][in_hi] - LO

        # onehotT [113, NWIN*cap]: row 0 = d2, rows 1..112 one-hot of rel
        oT = np.zeros((W + 1, nslots), bf16)
        oT[0, sl] = d2_s[m].astype(bf16)
        oT[1 + rel_s[m], sl] = 1

        # oht [128, NWIN*nch*W]: per chunk [e, w] one-hot
        ohtv = np.zeros((CHUNK, NWIN * nch * W), bf16)
        ch = sl // CHUNK          # global chunk index within core
        ohtv[sl % CHUNK, ch * W + rel_s[m]] = 1

        # dx stream [128, NWIN*nch*3]
        dxp = np.zeros((CHUNK, NWIN * nch, 3), f32)
        dxp[sl % CHUNK, ch] = dx_s[m]

        # x window [112, NWIN, 3]
        xw = np.zeros((W, NWIN, 3), f32)
        lo = c * OWN
        nrows = max(0, min(N - lo, OWN))
        xv = np.zeros((OWN, 3), f32)
        xv[:nrows] = np.asarray(x, f32)[lo:lo + nrows]
        xw[:, :, :] = xv.reshape(NWIN, W, 3).transpose(1, 0, 2)

        h_own = np.zeros((OWN, H), bf16)
        h_own[:nrows] = np.asarray(h, f32)[lo:lo + nrows].astype(bf16)

        data.append({
            "idx_lo": _wrap16(idx_lo), "idx_hi": _wrap16(idx_hi),
            "onehotT": oT,
            "oht": ohtv,
            "dxp": np.ascontiguousarray(
                dxp.reshape(CHUNK, NWIN * nch * 3)),
            "x_win": np.ascontiguousarray(xw),
            "h_own": h_own,
        })

    h_full = np.zeros((NP_FULL, H), bf16)
    h_full[:N] = np.asarray(h, f32).astype(bf16)
    sm = {"cap_lo": cap_lo, "cap_hi": cap_hi}
    if globals().get("DBG"):
        sm["dbg"] = 1
    return data, sm, h_full


def _wrap16(idx):
    w = idx.reshape(-1, 16).T.astype(np.int16)
    return np.ascontiguousarray(np.tile(w, (8, 1)))


# ------------------------------------------------------------- graph builder --

def _build(sm):
    import concourse.bass as bass
    import concourse.bacc as bacc
    import concourse.mybir as mybir
    import concourse.tile as tile

    AF = mybir.ActivationFunctionType
    DT = mybir.dt
    ALU = mybir.AluOpType

    cap_lo, cap_hi = sm["cap_lo"], sm["cap_hi"]
    cap = cap_lo + cap_hi
    nch = cap // CHUNK
    nchl = cap_lo // CHUNK
    ntw = -(-nch // TPT)          # tiles per window
    tiles = [(t * TPT, min((t + 1) * TPT, nch)) for t in range(ntw)]

    nc = bacc.Bacc("TRN2", num_devices=NCORES)

    def din(name, shape, dt):
        return nc.dram_tensor(name, shape, dt, kind="ExternalInput").ap()

    h_full_d = din("h_full", [NP_FULL, H], DT.bfloat16)
    h_own_d = din("h_own", [OWN, H], DT.bfloat16)
    ilo_d = din("idx_lo", [128, NWIN * cap_lo // 16], DT.int16)
    ihi_d = din("idx_hi", [128, NWIN * cap_hi // 16], DT.int16)
    oT_d = din("onehotT", [W + 1, NWIN * cap], DT.bfloat16)
    oht_d = din("oht", [128, NWIN * nch * W], DT.bfloat16)
    dxp_d = din("dxp", [128, NWIN * nch * 3], DT.float32)
    xw_d = din("x_win", [W, NWIN, 3], DT.float32)
    W1_d = din("W1", [2 * H + 1, H], DT.float32)
    W2_d = din("W2", [H, H], DT.float32)
    W3_d = din("W3", [H, H], DT.float32)
    W4_d = din("W4", [H, 1], DT.float32)
    b1_d = din("b1", [H, 1], DT.float32)
    b2_d = din("b2", [H, 1], DT.float32)
    b3_d = din("b3", [H, 1], DT.float32)
    b4_d = din("b4", [H, 1], DT.float32)
    g1_d = din("g1", [H, 1], DT.float32)
    beta_d = din("beta1", [H, 1], DT.float32)
    out_d = nc.dram_tensor("out", [OWN, 3], DT.float32,
                           kind="ExternalOutput").ap()
    dbg_d = (nc.dram_tensor("dbg", [128, 10, 512], DT.float32,
                            kind="ExternalOutput").ap() if sm.get("dbg")
             else None)

    with tile.TileContext(nc) as tc:
        _pools = []

        def _mkpool(**kw):
            p = tc.alloc_tile_pool(**kw)
            _pools.append(p)
            return p

        con = _mkpool(name="con", bufs=1)
        zps = _mkpool(name="zps", bufs=3, space="PSUM")   # z1/z2/z3
        sps = _mkpool(name="sps", bufs=1, space="PSUM")   # window stats
        cps = _mkpool(name="cps", bufs=1, space="PSUM")   # scale cols
        nps = _mkpool(name="nps", bufs=2, space="PSUM")   # pane
        htp = _mkpool(name="htp", bufs=2)                 # hT slabs
        bsg = _mkpool(name="bsg", bufs=2)                 # B staging
        gbp = _mkpool(name="gbp", bufs=2)                 # BgT + idx
        wsp = _mkpool(name="wsp", bufs=2)                 # window streams
        s1p = _mkpool(name="s1p", bufs=2 * ntw)           # s1 tiles
        wkp = _mkpool(name="wkp", bufs=3)                 # work tiles
        stp = _mkpool(name="stp", bufs=2)                 # stats rows
        osp = _mkpool(name="osp", bufs=2)                 # out staging
        drp = _mkpool(name="drp", bufs=2, space="DRAM")   # rstd roundtrip
        adp = _mkpool(name="adp", bufs=1, space="DRAM")   # A/B scratch

        # ---- constants ----
        def load_cast(dram_ap, shape, name):
            t_f = con.tile(shape, DT.float32, tag=f"{name}_f")
            nc.sync.dma_start(t_f[:], dram_ap)
            t_b = con.tile(shape, DT.bfloat16, tag=name)
            nc.vector.tensor_copy(t_b[:], t_f[:])
            return t_b

        W1a = load_cast(W1_d[0:H, :], [H, H], "W1a")
        W1b = load_cast(W1_d[H:2 * H, :], [H, H], "W1b")
        w1c = load_cast(W1_d[2 * H:2 * H + 1, :], [1, H], "w1c")
        W3b = load_cast(W3_d[:, :], [H, H], "W3b")
        W4b = load_cast(W4_d[:, :], [H, 1], "W4b")
        W2b = load_cast(W2_d[:, :], [H, H], "W2b")
        betab = load_cast(beta_d[:, :], [H, 1], "betab")

        def load_col(dram_ap, name):
            t = con.tile([H, 1], DT.float32, tag=name)
            nc.sync.dma_start(t[:], dram_ap)
            return t

        b1c = load_col(b1_d[:, :], "b1c")
        b2c = load_col(b2_d[:, :], "b2c")
        b3c = load_col(b3_d[:, :], "b3c")
        b4c = load_col(b4_d[:, :], "b4c")
        g1c = load_col(g1_d[:, :], "g1c")
        W2f = con.tile([H, H], DT.float32, tag="W2f")
        nc.sync.dma_start(W2f[:], W2_d[:, :])
        W2g = con.tile([H, H], DT.bfloat16, tag="W2g")
        nc.vector.tensor_scalar_mul(W2g[:], W2f[:], g1c[:])

        onesc = con.tile([H, 1], DT.bfloat16, tag="onesc")
        nc.vector.memset(onesc[:], 1.0)
        ones1 = con.tile([1, 1], DT.bfloat16, tag="ones1")
        nc.vector.memset(ones1[:], 1.0)

        # negu = -colsum(W2g) [1, H]; b2p = W2^T beta + b2
        u_ps = zps.tile([1, H], DT.float32, space="PSUM", tag="z")
        nc.tensor.matmul(u_ps[:], lhsT=onesc[:], rhs=W2g[:],
                         start=True, stop=True)
        negu = con.tile([1, H], DT.bfloat16, tag="negu")
        nc.vector.tensor_scalar_mul(negu[:], u_ps[:], -1.0)

        bb_ps = zps.tile([1, H], DT.float32, space="PSUM", tag="z")
        nc.tensor.matmul(bb_ps[:], lhsT=betab[:], rhs=W2b[:],
                         start=True, stop=True)
        bb_row = con.tile([1, H], DT.bfloat16, tag="bb_row")
        nc.vector.tensor_copy(bb_row[:], bb_ps[:])
        bbT_ps = zps.tile([H, 1], DT.float32, space="PSUM", tag="z")
        nc.tensor.matmul(bbT_ps[:], lhsT=bb_row[:], rhs=ones1[:],
                         start=True, stop=True)
        b2p = con.tile([H, 1], DT.float32, tag="b2p")
        nc.vector.tensor_add(b2p[:], bbT_ps[:], b2c[:])

        zcol = con.tile([128, 1], DT.float32, tag="zcol")
        nc.vector.memset(zcol[:], 0.0)

        # identity [128, 128] bf16
        iotai = con.tile([128, 128], DT.int32, tag="iotai")
        nc.gpsimd.iota(iotai[:], pattern=[[1, 128]], base=0,
                       channel_multiplier=0)
        iotab = con.tile([128, 128], DT.bfloat16, tag="iotab")
        nc.vector.tensor_copy(iotab[:], iotai[:])
        iotap = con.tile([128, 1], DT.int32, tag="iotap")
        nc.gpsimd.iota(iotap[:], pattern=[[0, 1]], base=0,
                       channel_multiplier=1)
        iotapf = con.tile([128, 1], DT.float32, tag="iotapf")
        nc.vector.tensor_copy(iotapf[:], iotap[:])
        ident = con.tile([128, 128], DT.bfloat16, tag="ident")
        nc.vector.tensor_scalar(out=ident[:], in0=iotab[:], scalar1=iotapf[:],
                                scalar2=None, op0=ALU.is_equal)

        # selector columns for window stats: tile t -> col t
        sels = []
        for t in range(ntw):
            sel = con.tile([H, ntw], DT.bfloat16, tag=f"sel{t}")
            nc.vector.memset(sel[:], 0.0)
            nc.vector.memset(sel[:, t:t + 1], 1.0)
            sels.append(sel)

        dbg_sb = None
        if dbg_d is not None:
            dbg_sb = con.tile([128, 10, 512], DT.float32, tag="dbg_sb")
            nc.vector.memset(dbg_sb[:], 0.0)

        # x in SBUF [112, NWIN, 3]
        x_sb = con.tile([W, NWIN, 3], DT.float32, tag="x_sb")
        nc.sync.dma_start(x_sb[:], xw_d[:, :, :])

        # ---- pretransform: B = h_full @ W1b, A = h_own @ W1a ----
        B_dr = adp.tile([NP_FULL, H], DT.bfloat16, tag="B_dr")
        A_dr = adp.tile([OWN, H], DT.bfloat16, tag="A_dr")

        def pretransform(src_d, nrows, wmat, dst_dr):
            SLAB = 2048
            nslab = -(-nrows // SLAB)
            for s in range(nslab):
                r0 = s * SLAB
                rn = min(SLAB, nrows - r0)
                hT = htp.tile([128, SLAB], DT.bfloat16, tag="hT")
                nc.sync.dma_start_transpose(
                    hT[:, :rn], src_d[r0:r0 + rn, :])
                stg = bsg.tile([128, SLAB // 128, H], DT.bfloat16, tag="stg")
                for b in range(rn // 128):
                    ps = zps.tile([128, H], DT.float32, space="PSUM",
                                  tag="z")
                    nc.tensor.matmul(ps[:],
                                     lhsT=hT[:, b * 128:(b + 1) * 128],
                                     rhs=wmat[:], start=True, stop=True)
                    nc.vector.tensor_copy(stg[:, b, :], ps[:])
                nc.sync.dma_start(
                    dst_dr[r0:r0 + rn, :].rearrange("(q p) d -> p q d",
                                                    p=128),
                    stg[:, :rn // 128, :])

        pretransform(h_full_d, NP_FULL, W1b, B_dr)
        pretransform(h_own_d, OWN, W1a, A_dr)

        # ---- per-window pipeline ----
        def gather_win(w):
            bglo = gbp.tile([128, 1, cap_lo], DT.bfloat16, tag="bglo")
            it_lo = gbp.tile([128, cap_lo // 16], DT.int16, tag="it_lo")
            nc.sync.dma_start(
                it_lo[:], ilo_d[:, w * cap_lo // 16:(w + 1) * cap_lo // 16])
            nc.gpsimd.dma_gather(
                out_ap=bglo[:, :, :], in_ap=B_dr[0:LO, :],
                idxs_ap=it_lo[:], num_idxs=cap_lo, num_idxs_reg=cap_lo,
                elem_size=H, transpose=True, single_packet=False)
            bghi = gbp.tile([128, 1, max(cap_hi, 128)], DT.bfloat16,
                            tag="bghi")
            if cap_hi:
                it_hi = gbp.tile([128, cap_hi // 16], DT.int16, tag="it_hi")
                nc.sync.dma_start(
                    it_hi[:],
                    ihi_d[:, w * cap_hi // 16:(w + 1) * cap_hi // 16])
                nc.gpsimd.dma_gather(
                    out_ap=bghi[:, :, 0:cap_hi], in_ap=B_dr[LO:NP_FULL, :],
                    idxs_ap=it_hi[:], num_idxs=cap_hi, num_idxs_reg=cap_hi,
                    elem_size=H, transpose=True, single_packet=False)
            return (bglo, bghi)

        def load_win_streams(w):
            aext = wsp.tile([W + 1, H], DT.bfloat16, tag="aext")
            nc.vector.tensor_copy(aext[0:1, :], w1c[:])
            nc.sync.dma_start(aext[1:W + 1, :], A_dr[w * W:(w + 1) * W, :])
            oT = wsp.tile([W + 1, cap], DT.bfloat16, tag="oT")
            nc.sync.dma_start(oT[:], oT_d[:, w * cap:(w + 1) * cap])
            oh = wsp.tile([128, nch * W], DT.bfloat16, tag="oh")
            nc.sync.dma_start(oh[:], oht_d[:, w * nch * W:(w + 1) * nch * W])
            dxt = wsp.tile([128, nch, 3], DT.float32, tag="dxt")
            nc.sync.dma_start(
                dxt[:], dxp_d[:, w * nch * 3:(w + 1) * nch * 3].rearrange(
                    "p (c d) -> p c d", c=nch))
            return {"aext": aext, "oT": oT, "oh": oh, "dxt": dxt}

        def pass_a(w, bgt, ws, s1_tiles):
            ss_ps = sps.tile([ntw, 512], DT.float32, space="PSUM", tag="ssp")
            sq_ps = sps.tile([ntw, 512], DT.float32, space="PSUM", tag="sqp")
            for t, (c0, c1) in enumerate(tiles):
                e0, e1 = c0 * CHUNK, c1 * CHUNK
                ne = e1 - e0
                z1 = zps.tile([H, TPT * CHUNK], DT.float32, space="PSUM",
                              tag="z")
                nc.tensor.matmul(z1[:, :ne], lhsT=ws["aext"][:],
                                 rhs=ws["oT"][:, e0:e1],
                                 start=True, stop=False)
                bglo, bghi = bgt
                segs = []
                if e0 < cap_lo:
                    segs.append((bglo, e0, min(e1, cap_lo), e0))
                if e1 > cap_lo:
                    s0 = max(e0, cap_lo)
                    segs.append((bghi, s0 - cap_lo, e1 - cap_lo, s0))
                for k, (buf, s, e, dst) in enumerate(segs):
                    nc.tensor.matmul(z1[:, dst - e0:dst - e0 + (e - s)],
                                     lhsT=ident[:],
                                     rhs=buf[:, 0, s:e],
                                     start=False, stop=(k == len(segs) - 1))
                if dbg_sb is not None and w == DBGW and t == 0:
                    nc.vector.tensor_copy(dbg_sb[:, 0, :ne], z1[:, :ne])
                    nc.vector.tensor_copy(dbg_sb[:, 2, :ne],
                                          bgt[0][:, 0, e0:e1])
                    nc.vector.tensor_copy(dbg_sb[:, 3, :],
                                          bgt[1][:, 0, 0:512])
                s1 = s1p.tile([H, TPT * CHUNK], DT.bfloat16, tag="s1T")
                nc.scalar.activation(s1[:, :ne], z1[:, :ne], AF.Silu,
                                     bias=b1c[:])
                if dbg_sb is not None and w == DBGW and t == 0:
                    nc.vector.tensor_copy(dbg_sb[:, 1, :ne], s1[:, :ne])
                s1_tiles[t] = (s1, ne)
                sq = wkp.tile([H, TPT * CHUNK], DT.bfloat16, tag="sq")
                nc.vector.tensor_mul(sq[:, :ne], s1[:, :ne], s1[:, :ne])
                sel = sels[t]
                first = t == 0
                last = t == ntw - 1
                nc.tensor.matmul(ss_ps[:, :ne], lhsT=sel[:], rhs=s1[:, :ne],
                                 start=first, stop=last,
                                 skip_group_check=True)
                nc.tensor.matmul(sq_ps[:, :ne], lhsT=sel[:], rhs=sq[:, :ne],
                                 start=first, stop=last,
                                 skip_group_check=True)
            return ss_ps, sq_ps

        def row_math2(w, stats_ps):
            ss_ps, sq_ps = stats_ps
            muf = stp.tile([ntw, 512], DT.float32, tag="muf")
            nc.vector.tensor_scalar_mul(muf[:], ss_ps[:], 1.0 / H)
            mu2 = stp.tile([ntw, 512], DT.float32, tag="mu2")
            nc.vector.tensor_mul(mu2[:], muf[:], muf[:])
            var = stp.tile([ntw, 512], DT.float32, tag="var")
            nc.vector.tensor_scalar(out=var[:], in0=sq_ps[:],
                                    scalar1=1.0 / H, scalar2=EPS,
                                    op0=ALU.mult, op1=ALU.add)
            nc.vector.tensor_sub(var[:], var[:], mu2[:])
            sd = stp.tile([ntw, 512], DT.float32, tag="sd")
            nc.scalar.activation(sd[:], var[:], AF.Sqrt, bias=zcol[0:ntw, :])
            rstd = stp.tile([ntw, 512], DT.float32, tag="rstd")
            nc.vector.reciprocal(rstd[:], sd[:])
            murs = stp.tile([ntw, 512], DT.bfloat16, tag="murs")
            nc.vector.tensor_mul(murs[:], muf[:], rstd[:])
            rs16 = stp.tile([ntw, 512], DT.bfloat16, tag="rs16")
            nc.vector.tensor_copy(rs16[:], rstd[:])
            rs_dr = drp.tile([ntw, 512], DT.bfloat16, tag="rs_dr")
            nc.sync.dma_start(rs_dr[:], rs16[:])
            mu_dr = drp.tile([ntw, 512], DT.bfloat16, tag="mu_dr")
            nc.sync.dma_start(mu_dr[:], murs[:])
            murs_fl = stp.tile([1, ntw * 512], DT.bfloat16, tag="murs_fl")
            nc.sync.dma_start(
                murs_fl[:],
                mu_dr[:, :].rearrange("g e -> (g e)")[None, :])
            if dbg_sb is not None and w == DBGW:
                nc.vector.tensor_copy(dbg_sb[0:ntw, 7, :], ss_ps[:])
                nc.vector.tensor_copy(dbg_sb[0:ntw, 8, :], sq_ps[:])
                nc.vector.tensor_copy(dbg_sb[0:1, 9, :], murs_fl[0:1, 0:512])
            return murs_fl, rs_dr

        def pass_b(w, ws, s1_tiles, murs_fl, rs_dr):
            scp = cps.tile([128, nch], DT.float32, space="PSUM", tag="scp")
            pane = nps.tile([W, 3], DT.float32, space="PSUM", tag="pane")
            for t, (c0, c1) in enumerate(tiles):
                e0 = c0 * CHUNK
                ne = (c1 - c0) * CHUNK
                s1, _ = s1_tiles.pop(t)
                rb = wkp.tile([128, 512], DT.bfloat16, tag="rb")
                rb_src = bass.AP(
                    tensor=rs_dr[:].tensor, offset=rs_dr[:].offset + t * 512,
                    ap=[[0, 128], [1, ne]])
                nc.sync.dma_start(rb[:, :ne], rb_src)

                s1n = wkp.tile([128, 512], DT.bfloat16, tag="s1n")
                nc.vector.tensor_mul(s1n[:, :ne], s1[:, :ne], rb[:, :ne])
                z2 = zps.tile([H, TPT * CHUNK], DT.float32, space="PSUM",
                              tag="z")
                nc.tensor.matmul(z2[:, :ne], lhsT=W2g[:], rhs=s1n[:, :ne],
                                 start=True, stop=False)
                nc.tensor.matmul(z2[:, :ne], lhsT=negu[:],
                                 rhs=murs_fl[0:1, t * 512:t * 512 + ne],
                                 start=False, stop=True)
                if dbg_sb is not None and w == DBGW and t == 0:
                    nc.vector.tensor_copy(dbg_sb[:, 4, :ne], z2[:, :ne])
                s2 = wkp.tile([128, 512], DT.bfloat16, tag="s2")
                nc.scalar.activation(s2[:, :ne], z2[:, :ne], AF.Silu,
                                     bias=b2p[:])
                if dbg_sb is not None and w == DBGW and t == 0:
                    nc.vector.tensor_copy(dbg_sb[:, 5, :ne], s2[:, :ne])
                z3 = zps.tile([H, TPT * CHUNK], DT.float32, space="PSUM",
                              tag="z")
                nc.tensor.matmul(z3[:, :ne], lhsT=W3b[:], rhs=s2[:, :ne],
                                 start=True, stop=True)
                s3 = wkp.tile([128, 512], DT.bfloat16, tag="s3")
                nc.scalar.activation(s3[:, :ne], z3[:, :ne], AF.Silu,
                                     bias=b3c[:])
                if dbg_sb is not None and w == DBGW and t == 0:
                    nc.vector.tensor_copy(dbg_sb[:, 6, :ne], s3[:, :ne])
                for q in range(c0, c1):
                    qe = (q - c0) * CHUNK
                    nc.tensor.matmul(scp[:, q:q + 1],
                                     lhsT=s3[:, qe:qe + CHUNK],
                                     rhs=W4b[:], start=True, stop=True,
                                     skip_group_check=True)
            sc4 = wkp.tile([128, nch], DT.float32, tag="sc4")
            nc.vector.tensor_scalar(out=sc4[:], in0=scp[:], scalar1=b4c[:],
                                    scalar2=None, op0=ALU.add)
            vec = wkp.tile([128, nch, 3], DT.bfloat16, tag="vec")
            nc.vector.tensor_tensor(
                out=vec[:], in0=ws["dxt"][:],
                in1=sc4[:, :, None].to_broadcast([128, nch, 3]),
                op=ALU.mult)
            for q in range(nch):
                nc.tensor.matmul(pane[:], lhsT=ws["oh"][:, q * W:(q + 1) * W],
                                 rhs=vec[:, q, :],
                                 start=(q == 0), stop=(q == nch - 1))
            return pane

        OGRP = 8
        ostg = [None]

        def close_window(w, pane):
            g = w % OGRP
            if g == 0:
                ostg[0] = osp.tile([W, OGRP, 3], DT.float32, tag="ostg", name="ostg")
            nc.vector.tensor_add(ostg[0][:, g, :], pane[:], x_sb[:, w, :])
            if g == OGRP - 1:
                w0 = w - (OGRP - 1)
                nc.sync.dma_start(
                    out_d[w0 * W:(w0 + OGRP) * W, :].rearrange(
                        "(q p) d -> p q d", p=W),
                    ostg[0][:])

        # software pipeline: passA(w) ... passB(w-1)
        prev = None
        for w in range(NWIN):
            bgt = gather_win(w)
            ws = load_win_streams(w)
            s1_tiles = {}
            stats_ps = pass_a(w, bgt, ws, s1_tiles)
            murs_fl, rs_dr = row_math2(w, stats_ps)
            if prev is not None:
                pw, pws, ps1, pmu, prs = prev
                pane = pass_b(pw, pws, ps1, pmu, prs)
                close_window(pw, pane)
            prev = (w, ws, s1_tiles, murs_fl, rs_dr)
        pw, pws, ps1, pmu, prs = prev
        pane = pass_b(pw, pws, ps1, pmu, prs)
        close_window(pw, pane)

        if dbg_sb is not None:
            nc.sync.dma_start(dbg_d[:, :, :], dbg_sb[:])

        for _p in reversed(_pools):
            _p.release()

    nc.compile()
    return nc


_CACHE = {}


def _get_nc(sm):
    key = hashlib.sha256(repr(sorted(sm.items())).encode()).hexdigest()
    if key not in _CACHE:
        _CACHE[key] = _build(sm)
    return _CACHE[key]


# ------------------------------------------------------------------- entry --

def kernel(h, x, e, dx, d2, W1, b1, g1, beta1, W2, b2, W3, b3, W4, b4):
    from concourse import bass_utils

    h = np.asarray(h); x = np.asarray(x); e = np.asarray(e)
    dx = np.asarray(dx); d2 = np.asarray(d2)
    data, sm, h_full = _prepare(h, x, e, dx, d2)
    nc = _get_nc(sm)

    wmats = {
        "W1": np.asarray(W1, f32), "W2": np.asarray(W2, f32),
        "W3": np.asarray(W3, f32),
        "W4": np.asarray(W4, f32).reshape(H, 1),
        "b1": np.asarray(b1, f32).reshape(H, 1),
        "b2": np.asarray(b2, f32).reshape(H, 1),
        "b3": np.asarray(b3, f32).reshape(H, 1),
        "b4": np.full((H, 1), np.asarray(b4, f32).reshape(-1)[0], f32),
        "g1": np.asarray(g1, f32).reshape(H, 1),
        "beta1": np.asarray(beta1, f32).reshape(H, 1),
    }
    in_maps = []
    for c in range(NCORES):
        d = data[c]
        m = {"h_full": h_full, "h_own": d["h_own"],
             "idx_lo": d["idx_lo"], "idx_hi": d["idx_hi"],
             "onehotT": d["onehotT"], "oht": d["oht"], "dxp": d["dxp"],
             "x_win": d["x_win"]}
        m.update(wmats)
        in_maps.append(m)

    res = bass_utils.run_bass_kernel_spmd(nc, in_maps,
                                          core_ids=list(range(NCORES)),
                                          trace=TRACE, **TRACE_KW)
    kernel._last_result = res

    out = np.asarray(x, f32).copy()
    for c in range(NCORES):
        lo = c * OWN
        nrows = max(0, min(N - lo, OWN))
        if nrows > 0:
            out[lo:lo + nrows] = res.results[c]["out"][:nrows]
    return out.astype(np.float32)

